# revision 48
# baseline (speedup 1.0000x reference)
"""NNUE forward kernel for Trainium2, 8-core SPMD, batch-sharded,
sparsity-exploiting (embedding-gather formulation).

Reference computation (B=4096, I=40960, H=256):
    h_p = clip(x_p @ W_p.T + b_p, 0, 1)   for p in {1,2}
    out = concat(h1, h2) @ v + b2         -> (B,)

x_p rows are sparse binary (~30 active features of 40960), so
x_p @ W_p.T is an embedding-sum: h[b] = sum_{active f} W_p.T[f, :].

Per core (512 batch rows): for each 128-row tile and perspective,
DMA-gather the active rows of the bf16 table W_p.T (split into two
20480-row halves so indices fit int16; <=1024 idxs per dma_gather — the
SWDGE ucode wedges beyond that; 4 SWDGE queues so all four Q7 core
pairs generate descriptors in parallel), then reduce the gathered slots
into per-row sums on the PE with a per-tile 0/1 selector matrix S
(fp8): h_tile[128, 256] = S.T @ gathered. l1_bias is folded in as an
extra gathered slot with an all-ones S row. Epilogue (relu off PSUM,
fused min/dot with v, + b2) runs on the Vector engine. No collectives
(pure data parallel; batch-sharded).
"""

import numpy as np
import ml_dtypes

import concourse.bass as bass
import concourse.mybir as mybir
from concourse import bacc
from concourse.tile import TileContext
from concourse.bass_utils import run_bass_kernel_spmd

BATCH = 4096
INPUT_SIZE = 40960
HIDDEN = 256
N_CORES = 8
B_CORE = BATCH // N_CORES  # 512
N_TILES = B_CORE // 128  # 4
HALF = INPUT_SIZE // 2  # 20480 rows per table half (int16 index range)

BF16 = mybir.dt.bfloat16
F32 = mybir.dt.float32
F8 = mybir.dt.float8e4
I16 = mybir.dt.int16

NP_BF16 = ml_dtypes.bfloat16
NP_F8 = mybir.dt.np(F8)

_NC_CACHE = {}
SIM_SAFE = False  # pad with the zero row everywhere (CoreSim asserts on -1)


def _build(nh):
    """nh: padded gather count per (128-row tile, perspective, half)."""
    nhc = nh // 16  # idx columns (16-partition wrap)
    ncol = nh // 128  # gather output columns per half
    cc = 2 * ncol  # selector columns per (tile, persp)

    nc = bacc.Bacc(
        "TRN2", target_bir_lowering=False, debug=False, num_swdge_queues=4
    )

    tbl = [
        [
            nc.dram_tensor(f"t{p}{h}", [HALF + 2, HIDDEN], BF16, kind="ExternalInput")
            for h in range(2)
        ]
        for p in range(2)
    ]
    idxd = nc.dram_tensor("idx", [128, 16 * nhc], I16, kind="ExternalInput")
    cntd = nc.dram_tensor("cnt", [1, 16], mybir.dt.uint32, kind="ExternalInput")
    smatd = nc.dram_tensor("smat", [8, 128, cc * 128], F8, kind="ExternalInput")
    vd = nc.dram_tensor("v", [128, 2, HIDDEN], F32, kind="ExternalInput")
    b2d = nc.dram_tensor("b2", [128, 1], F32, kind="ExternalInput")
    outd = nc.dram_tensor("out", [128, N_TILES], F32, kind="ExternalOutput")

    with TileContext(nc) as tc:
        with (
            tc.tile_pool(name="consts", bufs=1) as consts,
            tc.tile_pool(name="gp", bufs=4) as gp,
            tc.tile_pool(name="psum", bufs=6, space="PSUM") as pp,
            tc.tile_pool(name="ep", bufs=2) as ep,
        ):
            idxt = consts.tile([128, 16, nhc], I16, tag="idx")
            nc.sync.dma_start(out=idxt[:, :, :], in_=idxd[:, :])
            # Per-core exact counts for each (tile,persp,half)'s second
            # gather chunk: trailing -1 idxs are skipped by the ucode (no
            # descriptors, no DMA); the count register must match exactly.
            cnt_t = consts.tile([1, 16], mybir.dt.uint32, tag="cnt")
            nc.sync.dma_start(out=cnt_t, in_=cntd[:, :])
            cregs = []
            for g in range(16):
                r = nc.gpsimd.alloc_register(f"cnt{g}")
                nc.gpsimd.reg_load(r, cnt_t[0:1, g : g + 1])
                cregs.append(r)
            v_t = consts.tile([128, 2, HIDDEN], F32, tag="v")
            nc.sync.dma_start(out=v_t, in_=vd[:, :, :])
            b2_t = consts.tile([128, 1], F32, tag="b2")
            nc.sync.dma_start(out=b2_t, in_=b2d[:, :])
            outst = consts.tile([128, N_TILES], F32, tag="outst")
            # Preload all selector matrices during the ramp (DMA is idle
            # while the gather ucode IRAM-loads); removes the per-iteration
            # S-upload dependency that gated the final matmuls.
            s_all = consts.tile([128, 2 * N_TILES, cc * 128], F8, tag="smat")
            for i in range(2 * N_TILES):
                nc.sync.dma_start(out=s_all[:, i, :], in_=smatd[i, :, :])

            acc0 = None
            for i in range(2 * N_TILES):
                t, p = i // 2, i % 2
                s_t = s_all[:, i, :]
                # HW SWDGE limit: >1024 idxs in one dma_gather wedges the
                # exec unit — chunk to <=1024 (8 output columns) per call.
                # One tile per chunk so matmuls can start as each chunk lands.
                colmap = []  # global column -> (chunk tile, col within chunk)
                q = 0
                for h in range(2):
                    for k0 in range(0, nh, 1024):
                        k1 = min(k0 + 1024, nh)
                        gt = gp.tile(
                            [128, (k1 - k0) // 128, HIDDEN], BF16, tag=f"g{q}"
                        )
                        nreg = (
                            cregs[2 * i + h]
                            if (k0 == 1024 and k1 == nh)
                            else k1 - k0
                        )
                        nc.gpsimd.dma_gather(
                            gt,
                            tbl[p][h][:, :],
                            idxt[:, 2 * i + h, k0 // 16 : k1 // 16],
                            k1 - k0,
                            nreg,
                            HIDDEN,
                            queue_num=q % 4,
                        )
                        colmap += [(gt, cix) for cix in range((k1 - k0) // 128)]
                        q += 1
                psum = pp.tile([128, HIDDEN], F32, tag="psum")
                for j in range(cc):
                    gt, cix = colmap[j]
                    nc.tensor.matmul(
                        psum,
                        lhsT=s_t[:, j * 128 : (j + 1) * 128],
                        rhs=gt[:, cix, :],
                        start=(j == 0),
                        stop=(j == cc - 1),
                    )
                # bias is folded into the PSUM via a dedicated bias slot
                # (all-ones S row x gathered b1 table row), so the epilogue is
                # relu straight off PSUM, then fused (min 1.0, * v).
                clr = ep.tile([128, HIDDEN], F32, tag="clr")
                nc.vector.tensor_scalar_max(clr, psum, 0.0)
                prod = ep.tile([128, HIDDEN], F32, tag="prod")
                nc.vector.scalar_tensor_tensor(
                    prod,
                    clr,
                    1.0,
                    v_t[:, p, :],
                    op0=mybir.AluOpType.min,
                    op1=mybir.AluOpType.mult,
                )
                if p == 0:
                    acc0 = ep.tile([128, 1], F32, tag="acc0")
                    nc.vector.tensor_reduce(
                        acc0, prod, axis=mybir.AxisListType.X, op=mybir.AluOpType.add
                    )
                else:
                    acc1 = ep.tile([128, 1], F32, tag="acc1")
                    nc.vector.tensor_reduce(
                        acc1, prod, axis=mybir.AxisListType.X, op=mybir.AluOpType.add
                    )
                    # out[:, t] = (acc0 + b2) + acc1
                    nc.vector.scalar_tensor_tensor(
                        outst[:, t : t + 1],
                        acc0,
                        b2_t,
                        acc1,
                        op0=mybir.AluOpType.add,
                        op1=mybir.AluOpType.add,
                    )


    nc.compile()
    return nc


def _prep(x1, x2, l1_weights, l1_biases, l2_weight, l2_bias):
    """Host-side: tables, per-core index lists + selector matrices."""
    wt = np.ascontiguousarray(
        l1_weights.astype(np.float32).transpose(0, 2, 1)
    )  # [2, I, H]
    tabs = {}
    for p in range(2):
        for h in range(2):
            tt = np.zeros((HALF + 2, HIDDEN), dtype=NP_BF16)
            tt[:HALF] = wt[p, h * HALF : (h + 1) * HALF].astype(NP_BF16)
            if h == 0:
                # row HALF+1 = l1 bias for this perspective (bias slot)
                tt[HALF + 1] = l1_biases[p].astype(NP_BF16)
            tabs[f"t{p}{h}"] = tt

    v_full = np.ascontiguousarray(
        np.broadcast_to(
            l2_weight.astype(np.float32).reshape(1, 2, HIDDEN), (128, 2, HIDDEN)
        )
    )
    b2_full = np.full((128, 1), float(np.asarray(l2_bias).reshape(-1)[0]), np.float32)

    xs = [np.asarray(x1), np.asarray(x2)]
    # (core, tile, persp, half) -> (rows, feats) of active entries
    active = []
    max_n = 0
    for c in range(N_CORES):
        per_core = []
        for i in range(2 * N_TILES):
            t, p = i // 2, i % 2
            blk = xs[p][c * B_CORE + t * 128 : c * B_CORE + (t + 1) * 128]
            r_all, f_all = np.nonzero(blk)
            for h in range(2):
                sel = (f_all >= h * HALF) & (f_all < (h + 1) * HALF)
                r, f = r_all[sel], f_all[sel] - h * HALF
                per_core.append((r.astype(np.int32), f.astype(np.int32)))
                max_n = max(max_n, len(r))
        active.append(per_core)

    nh = max(2048, -(-(max_n + 1) // 128) * 128)  # +1: bias slot in half 0
    nhc = nh // 16
    ncol = nh // 128
    cc = 2 * ncol

    in_maps = []
    for c in range(N_CORES):
        idx_arr = np.empty((16, 128, nhc), np.int16)
        smat = np.zeros((8, 128, cc * 128), NP_F8)
        cnt = np.full((1, 16), nh - 1024, np.uint32)
        for i in range(2 * N_TILES):
            for h in range(2):
                r, f = active[c][i * 2 + h]
                n = len(r)
                m = n + (1 if h == 0 else 0)  # real slots incl bias slot
                # Exact-count second chunk with trailing -1s. First 4
                # iterations write fresh physical buffers (gp bufs=4): pad
                # those fully with the zero row so no slot ever holds
                # uninitialized SBUF. (CoreSim NaN-poisons fresh tiles, so
                # SIM_SAFE keeps full padding for simulation runs.)
                if i < 4 or SIM_SAFE or m <= 1024:
                    idxv = np.full(nh, HALF, np.int16)
                else:
                    idxv = np.full(nh, -1, np.int16)
                    cnt[0, i * 2 + h] = m - 1024
                if h == 0:
                    # slot 0 = bias slot: gathers the b1 row; S row all-ones
                    idxv[0] = HALF + 1
                    idxv[1 : n + 1] = f
                    smat[i, 0, 0:128] = 1.0
                    j = np.arange(n) + 1
                else:
                    idxv[:n] = f
                    j = np.arange(n)
                idx_arr[i * 2 + h] = np.tile(idxv.reshape(nhc, 16).T, (8, 1))
                smat[i, j % 128, (j // 128 + h * ncol) * 128 + r] = 1.0
        in_map = dict(tabs)
        in_map.update(
            idx=np.ascontiguousarray(
                idx_arr.transpose(1, 0, 2).reshape(128, 16 * nhc)
            ),
            cnt=cnt,
            smat=smat,
            v=v_full,
            b2=b2_full,
        )
        in_maps.append(in_map)
    return nh, in_maps


def _run(x1, x2, l1_weights, l1_biases, l2_weight, l2_bias, trace=False):
    nh, in_maps = _prep(x1, x2, l1_weights, l1_biases, l2_weight, l2_bias)
    if nh not in _NC_CACHE:
        _NC_CACHE[nh] = _build(nh)
    nc = _NC_CACHE[nh]

    res = run_bass_kernel_spmd(
        nc, in_maps, core_ids=list(range(N_CORES)), trace=trace
    )
    out = np.concatenate(
        [
            np.ascontiguousarray(res.results[c]["out"].T).reshape(B_CORE)
            for c in range(N_CORES)
        ]
    )
    return out.astype(np.float32), res


def kernel(**inputs):
    out, _ = _run(**inputs)
    return out


def kernel_profiled(**inputs):
    _, res = _run(**inputs, trace=True)
    return res
